# revision 5
# baseline (speedup 1.0000x reference)
"""Trainium2 Bass kernel for the NTM-style scatter-memory module.

Sharding: mem_rows (R=16384) sharded 8 ways (2048 rows/core); batch kept
whole on every core.  Per core the kernel computes, fully SBUF-resident:

  write path (b-partition layout, tolerant precision -> bf16):
    sim = (beta/|v| * v) @ (mem_r/|mem_r|).T          [PE, bf16]
    e   = exp(sim)            (softmax numerator; the 1/Z cancels
                               through the power-law renormalisation)
    wc  = conv3(e)            [DVE, 2 scaled copies + 2 adds]
    t   = exp(gamma * ln(k_piv*wc + conv_b))          [ACT, fused scale]
    S_t = sum_r t             [free with ACT accum_out] -> 4KB AllReduce
    add/erase = t.T @ [v*invS_t/B | invS_t/B]         [PE, bf16]
    mem2 = mem*(1-erase) + add                        [DVE, fp32]

  read path (r-partition layout, full precision -> fp32r matmuls):
    logits.T = Wp_shard.T @ x.T                       [PE, fp32r]
    e_p = exp(logits + bp)                            [ACT, exact exp]
    outT_partial = [mem2 | 1].T @ e_p                 [PE, fp32r]
                   (row 64 = local softmax denominator S_p)

Host: tiny controller heads (x@Wv etc., 0.2% of FLOPs), input slicing,
and the final 8-way partial sum + division by S_p.
"""

import numpy as np
import ml_dtypes

import concourse.bass as bass
import concourse.bacc as bacc
import concourse.tile as tile
from concourse import mybir
from concourse.bass_utils import run_bass_kernel_spmd

F32 = mybir.dt.float32
F32R = mybir.dt.float32r
BF16 = mybir.dt.bfloat16
AOP = mybir.AluOpType
AFT = mybir.ActivationFunctionType

B, D, R, W = 1024, 256, 16384, 64
NCORES = 8
RS = R // NCORES          # 2048 mem rows per core
RBLK = RS // 128          # 16 r-blocks of 128
BT = B // 128             # 8 batch tiles of 128
EPS_REF = 1e-16           # reference eps; sum(a+eps) == sum(a) + R*eps


def _build_program():
    nc = bacc.Bacc("TRN2", target_bir_lowering=False, debug=False,
                   num_devices=NCORES)

    # ---- per-core kernel I/O ----
    vT_t = nc.dram_tensor("vT_t", [W, B], BF16, kind="ExternalInput")
    memT_t = nc.dram_tensor("memT_t", [W, RS + 2], BF16, kind="ExternalInput")
    gamma_b = nc.dram_tensor("gamma_b", [128, BT], F32, kind="ExternalInput")
    v_b = nc.dram_tensor("v_b", [B, W], F32, kind="ExternalInput")
    xT = nc.dram_tensor("xT", [D, B], F32R, kind="ExternalInput")
    wp = nc.dram_tensor("wp", [D, RS], F32R, kind="ExternalInput")
    bp_c = nc.dram_tensor("bp_c", [RS], F32, kind="ExternalInput")
    mem_c = nc.dram_tensor("mem_c", [RS, W], F32, kind="ExternalInput")
    kparams = nc.dram_tensor("kparams", [128, 4], F32, kind="ExternalInput")
    hmask = nc.dram_tensor("hmask", [128, 2], F32, kind="ExternalInput")
    outT = nc.dram_tensor("outT", [W + 1, B], F32, kind="ExternalOutput")

    with tile.TileContext(nc) as tc:
        with (
            tc.tile_pool(name="const", bufs=1) as const,
            tc.tile_pool(name="epool", bufs=2) as epool,
            tc.tile_pool(name="q0p", bufs=2) as q0p,
            tc.tile_pool(name="q1p", bufs=2) as q1p,
            tc.tile_pool(name="lwcp", bufs=2) as lwcp,
            tc.tile_pool(name="tpool", bufs=1) as tpool,
            tc.tile_pool(name="eppool", bufs=1) as eppool,
            tc.tile_pool(name="vexp", bufs=1) as vexp,
            tc.tile_pool(name="addp", bufs=2) as addp,
            tc.tile_pool(name="m2p", bufs=3) as m2p,
            tc.tile_pool(name="outp", bufs=1) as outp,
            tc.tile_pool(name="smalls", bufs=1) as smalls,
            tc.tile_pool(name="ps_sim", bufs=2, space="PSUM") as ps_sim,
            tc.tile_pool(name="ps_log", bufs=2, space="PSUM") as ps_log,
            tc.tile_pool(name="ps_add", bufs=2, space="PSUM") as ps_add,
            tc.tile_pool(name="ps_out", bufs=1, space="PSUM") as ps_out,
            tc.tile_pool(name="dram", bufs=1, space="DRAM") as dram,
        ):
            # ---- load constants / weights into SBUF ----
            sb_vT = const.tile([W, B], BF16)
            nc.sync.dma_start(sb_vT[:], vT_t[:])
            sb_memT = const.tile([W, RS + 2], BF16)
            nc.sync.dma_start(sb_memT[:], memT_t[:])
            sb_gamma = const.tile([128, BT], F32)
            nc.sync.dma_start(sb_gamma[:], gamma_b[:])
            sb_kp = const.tile([128, 4], F32)
            nc.sync.dma_start(sb_kp[:], kparams[:])
            sb_hm = const.tile([128, 2], F32)
            nc.sync.dma_start(sb_hm[:], hmask[:])
            sb_v = const.tile([128, BT, W], F32)
            nc.sync.dma_start(sb_v[:], v_b.ap().rearrange("(t p) w -> p t w", p=128))
            sb_mem = const.tile([128, RBLK, W], F32)
            nc.sync.dma_start(sb_mem[:], mem_c.ap().rearrange("(t p) w -> p t w", p=128))
            sb_bp = const.tile([128, RBLK], F32)
            nc.sync.dma_start(sb_bp[:], bp_c.ap().rearrange("(t p) -> p t", p=128))
            sb_xT = const.tile([128, 2, B], F32R)
            nc.sync.dma_start(sb_xT[:], xT.ap().rearrange("(t p) n -> p t n", p=128))
            sb_wp = const.tile([128, 2, RS], F32R)
            for kt in range(2):
                nc.sync.dma_start(sb_wp[:, kt, :],
                                  wp.ap().rearrange("(t p) n -> p t n", p=128)[:, kt, :])

            # S_t accumulator ([128, BT]; column j = b-tile j)
            st_loc = smalls.tile([128, BT], F32)
            st_glob = smalls.tile([128, BT], F32)
            inv_st = smalls.tile([128, BT], F32)

            t_tiles = []
            # ================= WRITE PATH (per batch tile) =================
            for j in range(BT):
                # sim -> PSUM in 4x512 chunks + one 2-wide halo chunk
                e_t = epool.tile([128, RS + 2], BF16, tag="e")
                for c in range(4):
                    ps = ps_sim.tile([128, 512], F32, tag="simps")
                    nc.tensor.matmul(ps[:], sb_vT[:, j * 128:(j + 1) * 128],
                                     sb_memT[:, 1 + 512 * c: 1 + 512 * (c + 1)])
                    nc.scalar.activation(e_t[:, 1 + 512 * c: 1 + 512 * (c + 1)],
                                         ps[:], AFT.Exp)
                ps_h = ps_sim.tile([128, 2], F32, tag="simps")
                nc.tensor.matmul(ps_h[:], sb_vT[:, j * 128:(j + 1) * 128],
                                 sb_memT[:, 0:(RS + 2):(RS + 1)])
                nc.scalar.activation(e_t[:, 0:(RS + 2):(RS + 1)], ps_h[:], AFT.Exp)
                # halo masking (zero at the global edges)
                nc.vector.tensor_scalar(e_t[:, 0:1], e_t[:, 0:1],
                                        sb_hm[:, 0:1], None, AOP.mult)
                nc.vector.tensor_scalar(e_t[:, RS + 1:RS + 2], e_t[:, RS + 1:RS + 2],
                                        sb_hm[:, 1:2], None, AOP.mult)

                # conv3 along r:  wc/k2 = (k0/k2) e_l + (k1/k2) e_c + e_r
                q0 = q0p.tile([128, RS], BF16, tag="q0")
                nc.vector.tensor_scalar(q0[:], e_t[:, 0:RS], sb_kp[:, 0:1], None, AOP.mult)
                q1 = q1p.tile([128, RS], BF16, tag="q1")
                nc.vector.tensor_scalar(q1[:], e_t[:, 1:RS + 1], sb_kp[:, 1:2], None, AOP.mult)
                nc.vector.tensor_tensor(q0[:], q0[:], q1[:], AOP.add)
                nc.vector.tensor_tensor(q0[:], q0[:], e_t[:, 2:RS + 2], AOP.add)

                # t = exp(gamma * ln(k2 * wc' + conv_b)); S_t via accum
                lwc = lwcp.tile([128, RS], F32, tag="lwc")
                nc.scalar.activation(lwc[:], q0[:], AFT.Ln,
                                     bias=sb_kp[:, 3:4], scale=sb_kp[:, 2:3])
                t_t = tpool.tile([128, RS], BF16, tag=f"t{j}")
                nc.scalar.activation(t_t[:], lwc[:], AFT.Exp,
                                     scale=sb_gamma[:, j:j + 1],
                                     accum_out=st_loc[:, j:j + 1])
                t_tiles.append(t_t)

            # ================= READ PATH: logits + e_p =================
            ep_tiles = []
            for i in range(RBLK):
                ep = eppool.tile([128, B], F32R, tag=f"ep{i}")
                for c in range(2):
                    ps = ps_log.tile([128, 512], F32, tag="logps")
                    for kt in range(2):
                        nc.tensor.matmul(
                            ps[:],
                            sb_wp[:, kt, i * 128:(i + 1) * 128],
                            sb_xT[:, kt, c * 512:(c + 1) * 512],
                            start=(kt == 0), stop=(kt == 1))
                    nc.scalar.activation(ep[:, c * 512:(c + 1) * 512], ps[:],
                                         AFT.Exp, bias=sb_bp[:, i:i + 1])
                ep_tiles.append(ep)

            # ================= S_t AllReduce (4KB) =================
            cc_in = dram.tile([128, BT], F32)
            cc_out = dram.tile([128, BT], F32)
            nc.sync.dma_start(cc_in[:], st_loc[:])
            nc.gpsimd.collective_compute(
                "AllReduce", AOP.add,
                replica_groups=[list(range(NCORES))],
                ins=[cc_in.opt()], outs=[cc_out.opt()])
            nc.sync.dma_start(st_glob[:], cc_out[:])
            # invS = 1 / (S_t + R*eps)
            nc.vector.tensor_scalar(st_glob[:], st_glob[:], R * EPS_REF, None, AOP.add)
            nc.vector.reciprocal(inv_st[:], st_glob[:])

            # v'ext[j] = [v_j * invS/B | invS/B]  (bf16)
            vext_tiles = []
            for j in range(BT):
                ve = vexp.tile([128, W + 1], BF16, tag=f"ve{j}")
                nc.vector.tensor_scalar(ve[:, 0:W], sb_v[:, j, :],
                                        inv_st[:, j:j + 1], 1.0 / B, AOP.mult, AOP.mult)
                nc.vector.tensor_scalar(ve[:, W:W + 1], inv_st[:, j:j + 1],
                                        1.0 / B, None, AOP.mult)
                vext_tiles.append(ve)

            # ============ add/erase matmul + mem2, then out matmul ============
            ps_o = ps_out.tile([W + 1, B], F32)
            for i in range(RBLK):
                ps_a = ps_add.tile([128, W + 1], F32, tag="addps")
                for j in range(BT):
                    nc.tensor.matmul(ps_a[:],
                                     t_tiles[j][:, i * 128:(i + 1) * 128],
                                     vext_tiles[j][:],
                                     start=(j == 0), stop=(j == BT - 1))
                add_sb = addp.tile([128, W + 1], F32, tag="addsb")
                nc.vector.tensor_copy(add_sb[:], ps_a[:])
                # mem2ext = [mem*(1-erase) + add | 1]
                m2 = m2p.tile([128, W + 1], F32R, tag="m2")
                one_m = addp.tile([128, 1], F32, tag="onem")
                nc.vector.tensor_scalar(one_m[:], add_sb[:, W:W + 1], -1.0, 1.0,
                                        AOP.mult, AOP.add)
                nc.vector.tensor_scalar(m2[:, 0:W], sb_mem[:, i, :], one_m[:],
                                        None, AOP.mult)
                nc.vector.tensor_tensor(m2[:, 0:W], m2[:, 0:W], add_sb[:, 0:W], AOP.add)
                nc.vector.tensor_scalar(m2[:, W:W + 1], one_m[:], 0.0, 1.0,
                                        AOP.mult, AOP.add)
                # outT += mem2ext.T @ e_p   (fp32r, accumulated over r-blocks)
                for c in range(2):
                    nc.tensor.matmul(ps_o[:, c * 512:(c + 1) * 512],
                                     m2[:],
                                     ep_tiles[i][:, c * 512:(c + 1) * 512],
                                     start=(i == 0), stop=(i == RBLK - 1))

            out_sb = outp.tile([W + 1, B], F32)
            for c in range(2):
                nc.vector.tensor_copy(out_sb[:, c * 512:(c + 1) * 512],
                                      ps_o[:, c * 512:(c + 1) * 512])
            nc.sync.dma_start(outT[:], out_sb[:])

    nc.compile()
    return nc


_NC_CACHE = []


def _get_program():
    if not _NC_CACHE:
        _NC_CACHE.append(_build_program())
    return _NC_CACHE[0]


def kernel(x, Wv, bv, Wb, bb, Wg, bg, Wp, bp, conv_k, conv_b, mem):
    x = np.asarray(x, np.float64)
    Wv = np.asarray(Wv, np.float64)
    bv = np.asarray(bv, np.float64)
    Wb = np.asarray(Wb, np.float64)
    bb = np.asarray(bb, np.float64)
    Wg = np.asarray(Wg, np.float64)
    bg = np.asarray(bg, np.float64)
    Wp32 = np.ascontiguousarray(np.asarray(Wp, np.float32))
    bp32 = np.asarray(bp, np.float32)
    ck = np.asarray(conv_k, np.float64).reshape(-1)
    cb = float(np.asarray(conv_b, np.float64).reshape(-1)[0])
    mem64 = np.asarray(mem, np.float64)
    mem32 = np.asarray(mem, np.float32)

    # ---- controller heads on host (0.2% of total FLOPs) ----
    v = x @ Wv + bv                                   # [B, W]
    beta = np.log1p(np.exp(x @ Wb + bb))              # [B, 1] softplus
    gamma = 1.0 + np.log1p(np.exp(x @ Wg + bg))       # [B, 1]
    vn = np.linalg.norm(v, axis=-1, keepdims=True)    # [B, 1]
    mn = np.linalg.norm(mem64, axis=-1)               # [R]

    vT_t = np.ascontiguousarray((v * (beta / vn)).T.astype(ml_dtypes.bfloat16))
    gamma_b = np.ascontiguousarray(
        gamma.reshape(BT, 128).T.astype(np.float32))
    v_b32 = np.ascontiguousarray(v.astype(np.float32))
    xT32 = np.ascontiguousarray(np.asarray(x, np.float32).T)

    k0, k1, k2 = ck
    kparams = np.tile(
        np.array([k0 / k2, k1 / k2, k2, cb], np.float32), (128, 1))

    in_maps = []
    for c in range(NCORES):
        lo, hi = c * RS, (c + 1) * RS
        # shard rows with one halo row on each side (edges clamped+masked)
        lo_h, hi_h = max(lo - 1, 0), min(hi + 1, R)
        sh = np.empty((RS + 2, W), np.float64)
        sh[1:RS + 1] = mem64[lo:hi]
        sh[0] = mem64[lo_h]
        sh[RS + 1] = mem64[hi_h - 1]
        shn = np.empty((RS + 2,), np.float64)
        shn[1:RS + 1] = mn[lo:hi]
        shn[0] = mn[lo_h]
        shn[RS + 1] = mn[hi_h - 1]
        memT_t = np.ascontiguousarray(
            (sh / shn[:, None]).T.astype(ml_dtypes.bfloat16))
        hmask = np.tile(np.array(
            [1.0 if c > 0 else 0.0, 1.0 if c < NCORES - 1 else 0.0],
            np.float32), (128, 1))
        in_maps.append({
            "vT_t": vT_t,
            "memT_t": memT_t,
            "gamma_b": gamma_b,
            "v_b": v_b32,
            "xT": xT32,
            "wp": np.ascontiguousarray(Wp32[:, lo:hi]),
            "bp_c": np.ascontiguousarray(bp32[lo:hi]),
            "mem_c": np.ascontiguousarray(mem32[lo:hi]),
            "kparams": kparams,
            "hmask": hmask,
        })

    nc = _get_program()
    global _last_in_maps
    _last_in_maps = in_maps
    res = run_bass_kernel_spmd(nc, in_maps, list(range(NCORES)))

    acc = np.zeros((W + 1, B), np.float64)
    for c in range(NCORES):
        acc += np.asarray(res.results[c]["outT"], np.float64)
    out = (acc[:W] / acc[W]).T
    return np.ascontiguousarray(out.astype(np.float32))
